# revision 3
# baseline (speedup 1.0000x reference)
"""Trainium2 Bass kernel for nn_AttentionBlock (B=4, C=512, S=2048, K=V=512).

Reference computation (per batch b):
  xb = x[b]                       # [C, S] channel-first
  q = xb.T @ Wq + bq              # [S, K]
  k = xb.T @ Wk + bk
  v = xb.T @ Wv + bv
  s = q @ k.T / sqrt(K)           # [Sq, Sk], causal mask j<=i
  p = softmax(s, axis=QUERY)      # normalize over i for each column j
  act = (p @ v).T                 # [V, S]
  out = concat([xb, act], axis=0) # [C+V, S]

Sharding: 8 cores = 4 batches x 2 "parity" shards. Core (b, par) owns the
interleaved key-tiles t = 2u+par (u=0..7, 128 keys each). Column-softmax
(over queries i) is fully local to a key j, so each core computes complete
softmax columns for its keys and a partial activation
  AT_par[v, q] = sum_{its j} P[j, q] * V[j, v]
The two partials per batch are summed on the host during unshard.

All on-device tensors are kept feature-major so the whole pipeline needs
zero transposes:
  QT[d, i] = Wq.T @ xb        (lhsT=Wq[c,d], rhs=xb[c,i])
  KT[d, j] = Wk.T @ xkv       (xkv = host-gathered key columns of xb)
  V [j, v] = xkv.T @ Wv       (lhsT=xkv[c,j], rhs=Wv[c,v])
  ST[j, i] = KT.T @ QT        -> softmax along the FREE axis (i) per row j
  AT[v, q] = W.T @ E          (W = V scaled by 1/Z per row j, E = exp scores)

The causal structure is identical for both parities (same chunk counts per
u), so one static program serves all 8 cores; the parity difference lives
entirely in the data (xkv gather + the two diagonal mask tiles).

Matmuls run as float32r (full PE rate, ~1e-6 rel err); fp32r operands are
produced by rounding compute ops (DMA-sourced data gets one rounding copy).
"""

import math

import numpy as np

B, C, S = 4, 512, 2048
KEY = 512
VAL = 512
NU = 8          # key-tiles (128 wide) per core
NCH = 4         # 512-wide i/q chunks
RS = 1.0 / math.sqrt(KEY)

_CACHE = {}


def _build_module():
    import concourse.bass as bass
    import concourse.tile as tile
    from concourse import bacc, mybir

    F32 = mybir.dt.float32
    F32R = mybir.dt.float32r
    AF = mybir.ActivationFunctionType
    ALU = mybir.AluOpType
    ts = bass.ts

    nc = bacc.Bacc("TRN2", target_bir_lowering=False, debug=False,
                   enable_asserts=False, num_devices=8)

    x_d = nc.dram_tensor("x", [C, S], F32, kind="ExternalInput").ap()
    xkv_d = nc.dram_tensor("xkv", [C, NU * 128], F32, kind="ExternalInput").ap()
    wq_d = nc.dram_tensor("wq", [C, KEY], F32, kind="ExternalInput").ap()
    wk_d = nc.dram_tensor("wk", [C, KEY], F32, kind="ExternalInput").ap()
    wv_d = nc.dram_tensor("wv", [C, VAL], F32, kind="ExternalInput").ap()
    bq_d = nc.dram_tensor("bq", [KEY], F32, kind="ExternalInput").ap()
    bk_d = nc.dram_tensor("bk", [KEY], F32, kind="ExternalInput").ap()
    bv_d = nc.dram_tensor("bv", [VAL], F32, kind="ExternalInput").ap()
    me_d = nc.dram_tensor("maskE", [128, 512], F32, kind="ExternalInput").ap()
    mo_d = nc.dram_tensor("maskO", [128, 512], F32, kind="ExternalInput").ap()
    at_d = nc.dram_tensor("at", [VAL, S], F32, kind="ExternalOutput").ap()

    with tile.TileContext(nc) as tc:
        with tc.tile_pool(name="stage", bufs=3) as stage, \
             tc.tile_pool(name="ring", bufs=20) as ring, \
             tc.tile_pool(name="persist", bufs=1) as persist, \
             tc.tile_pool(name="outp", bufs=4) as outp, \
             tc.tile_pool(name="psum", bufs=4, space="PSUM") as psum:

            # ---- small constants -------------------------------------
            masks = {}
            for nm, d in (("E", me_d), ("O", mo_d)):
                m = persist.tile([128, 512], F32, name=f"mask{nm}", tag=f"mask{nm}")
                nc.sync.dma_start(m[:], d)
                masks[nm] = m
            bqs, bks = [], []
            for dt_i in range(4):
                bq_t = persist.tile([128, 1], F32, name=f"bq{dt_i}", tag=f"bq{dt_i}")
                nc.sync.dma_start(bq_t[:], bq_d[ts(dt_i, 128)][:, None])
                bqs.append(bq_t)
                bk_t = persist.tile([128, 1], F32, name=f"bk{dt_i}", tag=f"bk{dt_i}")
                nc.sync.dma_start(bk_t[:], bk_d[ts(dt_i, 128)][:, None])
                bks.append(bk_t)
            bvb = persist.tile([128, 512], F32, name="bvb", tag="bvb")
            bv_bcast = bass.AP(tensor=bv_d.tensor, offset=bv_d.offset,
                               ap=[[0, 128]] + list(bv_d.ap))
            nc.gpsimd.dma_start(bvb[:], bv_bcast)

            # ---- weights: stage + round to f32r ----------------------
            wr = {}
            for nm, d in (("q", wq_d), ("k", wk_d), ("v", wv_d)):
                for ct in range(4):
                    st = stage.tile([128, 512], F32, name=f"wst_{nm}{ct}", tag="w_st")
                    nc.sync.dma_start(st[:], d[ts(ct, 128), :])
                    r = persist.tile([128, 512], F32R, name=f"w_{nm}{ct}",
                                     tag=f"w_{nm}{ct}")
                    nc.gpsimd.tensor_copy(r[:], st[:])
                    wr[nm, ct] = r

            # ---- xkv: stage + round ----------------------------------
            xkvr = []
            for ct in range(4):
                st = stage.tile([128, 1024], F32, name=f"xkvst{ct}", tag="xkv_st")
                nc.sync.dma_start(st[:], xkv_d[ts(ct, 128), :])
                r = persist.tile([128, 1024], F32R, name=f"xkv{ct}", tag=f"xkv{ct}")
                nc.vector.tensor_copy(r[:], st[:])
                xkvr.append(r)

            # ---- QT[d, i] --------------------------------------------
            qt = [persist.tile([128, 2048], F32R, name=f"qt{d}", tag=f"qt{d}")
                  for d in range(4)]
            for ic in range(NCH):
                xr_ic = []
                for ct in range(4):
                    st = stage.tile([128, 512], F32, name=f"xst_{ic}_{ct}", tag="x_st")
                    nc.sync.dma_start(st[:], x_d[ts(ct, 128), ts(ic, 512)])
                    r = ring.tile([128, 512], F32R, name=f"xr_{ic}_{ct}", tag="r2k")
                    nc.vector.tensor_copy(r[:], st[:])
                    xr_ic.append(r)
                for dt_i in range(4):
                    ps = psum.tile([128, 512], F32, name=f"ps_qt{ic}{dt_i}", tag="ps")
                    for ct in range(4):
                        nc.tensor.matmul(ps[:], wr["q", ct][:, ts(dt_i, 128)],
                                         xr_ic[ct][:],
                                         start=(ct == 0), stop=(ct == 3))
                    nc.scalar.activation(qt[dt_i][:, ts(ic, 512)], ps[:],
                                         AF.Identity, bias=bqs[dt_i][:], scale=1.0)

            # ---- KT[d, j] --------------------------------------------
            kt = [persist.tile([128, 1024], F32R, name=f"kt{d}", tag=f"kt{d}")
                  for d in range(4)]
            for jc in range(2):
                for dt_i in range(4):
                    ps = psum.tile([128, 512], F32, name=f"ps_kt{jc}{dt_i}", tag="ps")
                    for ct in range(4):
                        nc.tensor.matmul(ps[:], wr["k", ct][:, ts(dt_i, 128)],
                                         xkvr[ct][:, ts(jc, 512)],
                                         start=(ct == 0), stop=(ct == 3))
                    nc.scalar.activation(kt[dt_i][:, ts(jc, 512)], ps[:],
                                         AF.Identity, bias=bks[dt_i][:], scale=1.0)

            # ---- V[j, v] (+bv) ---------------------------------------
            vw = []
            for u in range(NU):
                ps = psum.tile([128, 512], F32, name=f"ps_v{u}", tag="ps")
                for ct in range(4):
                    nc.tensor.matmul(ps[:], xkvr[ct][:, ts(u, 128)], wr["v", ct][:],
                                     start=(ct == 0), stop=(ct == 3))
                t = persist.tile([128, 512], F32R, name=f"vw{u}", tag=f"vw{u}")
                nc.vector.tensor_add(t[:], ps[:], bvb[:])
                vw.append(t)

            # ---- scores + column softmax, PV interleaved -------------
            ep = {}

            def do_S(u):
                c0 = u // 2
                stats = persist.tile([128, 4], F32, name=f"stats{u}", tag=f"stats{u}")
                for c in range(c0, NCH):
                    ps = psum.tile([128, 512], F32, name=f"ps_s{u}{c}", tag="ps")
                    for dt_i in range(4):
                        nc.tensor.matmul(ps[:], kt[dt_i][:, ts(u, 128)],
                                         qt[dt_i][:, ts(c, 512)],
                                         start=(dt_i == 0), stop=(dt_i == 3))
                    e = ring.tile([128, 512], F32R, name=f"e_{u}_{c}", tag="r2k")
                    ep[u, c] = e
                    if c == c0:
                        # additive causal mask (0 valid / -1e9 masked) applied
                        # to the scores in PSUM; exp then underflows to 0
                        m = masks["E" if u % 2 == 0 else "O"]
                        nc.vector.tensor_add(ps[:], ps[:], m[:])
                    nc.scalar.activation(e[:], ps[:], AF.Exp, bias=0.0, scale=RS,
                                         accum_out=stats[:, c:c + 1])
                zs = persist.tile([128, 1], F32, name=f"zs{u}", tag=f"zs{u}")
                nc.vector.reduce_sum(zs[:], stats[:, c0:NCH],
                                     axis=mybir.AxisListType.X)
                zi = persist.tile([128, 1], F32, name=f"zi{u}", tag=f"zi{u}")
                nc.vector.reciprocal(zi[:], zs[:])
                nc.vector.tensor_scalar_mul(vw[u][:], vw[u][:], zi[:])

            def do_PV(c):
                n_u = min(2 * c + 2, NU)
                for vb in range(4):
                    ps = psum.tile([128, 512], F32, name=f"ps_pv{c}{vb}", tag="ps")
                    for u in range(n_u):
                        nc.tensor.matmul(ps[:], vw[u][:, ts(vb, 128)], ep[u, c][:],
                                         start=(u == 0), stop=(u == n_u - 1))
                    o = outp.tile([128, 512], F32, name=f"o_{c}_{vb}", tag="o")
                    nc.scalar.copy(o[:], ps[:])
                    nc.sync.dma_start(at_d[ts(vb, 128), ts(c, 512)], o[:])

            do_S(0)
            do_S(1)
            do_S(2)
            do_PV(0)
            do_S(3)
            do_S(4)
            do_PV(1)
            do_S(5)
            do_S(6)
            do_PV(2)
            do_S(7)
            do_PV(3)

    nc.compile()
    return nc


def _get_module():
    if "nc" not in _CACHE:
        _CACHE["nc"] = _build_module()
    return _CACHE["nc"]


def _host_masks(par):
    # additive masks: 0.0 where valid (i >= j), -1e9 where masked
    p = np.arange(128)[:, None]
    f = np.arange(512)[None, :]
    mE = np.where(f >= p + 128 * par, 0.0, -1e9).astype(np.float32)
    mO = np.where(f >= p + 256 + 128 * par, 0.0, -1e9).astype(np.float32)
    return mE, mO


def kernel(x, Wq, bq, Wk, bk, Wv, bv):
    from concourse.bass_utils import run_bass_kernel_spmd

    x = np.ascontiguousarray(np.asarray(x, dtype=np.float32))
    Wq = np.ascontiguousarray(np.asarray(Wq, dtype=np.float32))
    Wk = np.ascontiguousarray(np.asarray(Wk, dtype=np.float32))
    Wv = np.ascontiguousarray(np.asarray(Wv, dtype=np.float32))
    bq = np.ascontiguousarray(np.asarray(bq, dtype=np.float32))
    bk = np.ascontiguousarray(np.asarray(bk, dtype=np.float32))
    bv = np.ascontiguousarray(np.asarray(bv, dtype=np.float32))

    nc = _get_module()

    in_maps = []
    for b in range(B):
        for par in (0, 1):
            cols = np.concatenate(
                [np.arange(128 * (2 * u + par), 128 * (2 * u + par) + 128)
                 for u in range(NU)])
            mE, mO = _host_masks(par)
            in_maps.append({
                "x": x[b],
                "xkv": np.ascontiguousarray(x[b][:, cols]),
                "wq": Wq, "wk": Wk, "wv": Wv,
                "bq": bq, "bk": bk, "bv": bv,
                "maskE": mE, "maskO": mO,
            })

    res = run_bass_kernel_spmd(nc, in_maps, core_ids=list(range(8)))
    _CACHE["last_results"] = res

    act = np.empty((B, VAL, S), dtype=np.float32)
    for b in range(B):
        act[b] = res.results[2 * b]["at"] + res.results[2 * b + 1]["at"]
    return np.concatenate([x, act], axis=1)


# revision 4
# speedup vs baseline: 1.3718x; 1.3718x over previous
"""Trainium2 Bass kernel for nn_AttentionBlock (B=4, C=512, S=2048, K=V=512).

Reference computation (per batch b):
  xb = x[b]                       # [C, S] channel-first
  q = xb.T @ Wq + bq              # [S, K]
  k = xb.T @ Wk + bk
  v = xb.T @ Wv + bv
  s = q @ k.T / sqrt(K)           # [Sq, Sk], causal mask j<=i
  p = softmax(s, axis=QUERY)      # normalize over i for each column j
  act = (p @ v).T                 # [V, S]
  out = concat([xb, act], axis=0) # [C+V, S]

Sharding: 8 cores = 4 batches x 2 "parity" shards. Core (b, par) owns the
interleaved key-tiles t = 2u+par (u=0..7, 128 keys each). Column-softmax
(over queries i) is fully local to a key j, so each core computes complete
softmax columns for its keys and a partial activation
  AT_par[v, q] = sum_{its j} P[j, q] * V[j, v]
The two partials per batch are summed on the host during unshard.

All on-device tensors are kept feature-major so the whole pipeline needs
zero transposes:
  QT[d, i] = Wq.T @ xb        (lhsT=Wq[c,d], rhs=xb[c,i])
  KT[d, j] = Wk.T @ xkv       (xkv = host-gathered key columns of xb)
  V [j, v] = xkv.T @ Wv       (lhsT=xkv[c,j], rhs=Wv[c,v])
  ST[j, i] = KT.T @ QT        -> softmax along the FREE axis (i) per row j
  AT[v, q] = W.T @ E          (W = V scaled by 1/Z per row j, E = exp scores)

The causal structure is identical for both parities (same chunk counts per
u), so one static program serves all 8 cores; the parity difference lives
entirely in the data (xkv gather + the two additive diagonal mask tiles).

Matmul operands are fp16 (full PE rate + fast weight load); inputs are cast
to fp16 on the host, accumulation stays fp32 in PSUM, softmax statistics in
fp32. Measured end-to-end absmax error ~2e-3 (same order as an fp32r build).
"""

import math

import numpy as np

B, C, S = 4, 512, 2048
KEY = 512
VAL = 512
NU = 8          # key-tiles (128 wide) per core
NCH = 4         # 512-wide i/q chunks
RS = 1.0 / math.sqrt(KEY)

_CACHE = {}


def _build_module():
    import concourse.bass as bass
    import concourse.tile as tile
    from concourse import bacc, mybir

    F32 = mybir.dt.float32
    F16 = mybir.dt.float16
    AF = mybir.ActivationFunctionType
    ts = bass.ts

    nc = bacc.Bacc("TRN2", target_bir_lowering=False, debug=False,
                   enable_asserts=False, num_devices=8)

    x_d = nc.dram_tensor("x16", [C, S], F16, kind="ExternalInput").ap()
    xkv_d = nc.dram_tensor("xkv16", [C, NU * 128], F16, kind="ExternalInput").ap()
    wq_d = nc.dram_tensor("wq16", [C, KEY], F16, kind="ExternalInput").ap()
    wk_d = nc.dram_tensor("wk16", [C, KEY], F16, kind="ExternalInput").ap()
    wv_d = nc.dram_tensor("wv16", [C, VAL], F16, kind="ExternalInput").ap()
    bq_d = nc.dram_tensor("bq", [KEY], F32, kind="ExternalInput").ap()
    bk_d = nc.dram_tensor("bk", [KEY], F32, kind="ExternalInput").ap()
    bv_d = nc.dram_tensor("bv", [VAL], F32, kind="ExternalInput").ap()
    me_d = nc.dram_tensor("maskE", [128, 512], F32, kind="ExternalInput").ap()
    mo_d = nc.dram_tensor("maskO", [128, 512], F32, kind="ExternalInput").ap()
    at_d = nc.dram_tensor("at", [VAL, S], F32, kind="ExternalOutput").ap()

    # dram views with the 512-row axis split into 4 partition tiles
    x_v = x_d.rearrange("(ct p) s -> p ct s", p=128)
    xkv_v = xkv_d.rearrange("(ct p) s -> p ct s", p=128)
    wq_v = wq_d.rearrange("(ct p) d -> p ct d", p=128)
    wk_v = wk_d.rearrange("(ct p) d -> p ct d", p=128)
    wv_v = wv_d.rearrange("(ct p) d -> p ct d", p=128)
    bq_v = bq_d.rearrange("(dt p) -> p dt", p=128)
    bk_v = bk_d.rearrange("(dt p) -> p dt", p=128)

    with tile.TileContext(nc) as tc:
        with tc.tile_pool(name="persist", bufs=1) as persist, \
             tc.tile_pool(name="ering", bufs=20) as ering, \
             tc.tile_pool(name="outp", bufs=4) as outp, \
             tc.tile_pool(name="psum", bufs=4, space="PSUM") as psum:

            # ---- inputs: weights/xkv on scalar queue, x on sync queue ----
            w16 = {}
            for nm, v in (("k", wk_v), ("v", wv_v), ("q", wq_v)):
                t = persist.tile([128, 4, 512], F16, name=f"w{nm}", tag=f"w{nm}")
                nc.scalar.dma_start(t[:], v)
                w16[nm] = t
            xkv16 = persist.tile([128, 4, 1024], F16, name="xkv16s", tag="xkv")
            for jc in range(2):
                nc.sync.dma_start(xkv16[:, :, ts(jc, 512)], xkv_v[:, :, ts(jc, 512)])
            x16 = []
            for ic in range(NCH):
                t = persist.tile([128, 4, 512], F16, name=f"x16_{ic}", tag=f"x{ic}")
                nc.sync.dma_start(t[:], x_v[:, :, ts(ic, 512)])
                x16.append(t)

            masks = {}
            for nm, d in (("E", me_d), ("O", mo_d)):
                m = persist.tile([128, 512], F32, name=f"mask{nm}", tag=f"mask{nm}")
                nc.scalar.dma_start(m[:], d)
                masks[nm] = m
            bq_sb = persist.tile([128, 4], F32, name="bq_sb", tag="bq_sb")
            nc.scalar.dma_start(bq_sb[:], bq_v)
            bk_sb = persist.tile([128, 4], F32, name="bk_sb", tag="bk_sb")
            nc.scalar.dma_start(bk_sb[:], bk_v)
            bvb = persist.tile([128, 512], F32, name="bvb", tag="bvb")
            bv_bcast = bass.AP(tensor=bv_d.tensor, offset=bv_d.offset,
                               ap=[[0, 128]] + list(bv_d.ap))
            nc.gpsimd.dma_start(bvb[:], bv_bcast)

            # ---- KT[d, j] ------------------------------------------------
            kt = [persist.tile([128, 1024], F16, name=f"kt{d}", tag=f"kt{d}")
                  for d in range(4)]
            for jc in range(2):
                for dt_i in range(4):
                    ps = psum.tile([128, 512], F32, name=f"ps_kt{jc}{dt_i}", tag="ps")
                    for ct in range(4):
                        nc.tensor.matmul(ps[:], w16["k"][:, ct, ts(dt_i, 128)],
                                         xkv16[:, ct, ts(jc, 512)],
                                         start=(ct == 0), stop=(ct == 3))
                    nc.vector.tensor_scalar_add(kt[dt_i][:, ts(jc, 512)], ps[:],
                                                bk_sb[:, dt_i:dt_i + 1])

            # ---- V[j, v] (+bv) ------------------------------------------
            vw = []
            for u in range(NU):
                ps = psum.tile([128, 512], F32, name=f"ps_v{u}", tag="ps")
                for ct in range(4):
                    nc.tensor.matmul(ps[:], xkv16[:, ct, ts(u, 128)],
                                     w16["v"][:, ct, :],
                                     start=(ct == 0), stop=(ct == 3))
                t = persist.tile([128, 512], F16, name=f"vw{u}", tag=f"vw{u}")
                nc.vector.tensor_add(t[:], ps[:], bvb[:])
                vw.append(t)

            # ---- QT[d, i] ------------------------------------------------
            qt = [persist.tile([128, 2048], F16, name=f"qt{d}", tag=f"qt{d}")
                  for d in range(4)]
            for ic in range(NCH):
                for dt_i in range(4):
                    ps = psum.tile([128, 512], F32, name=f"ps_qt{ic}{dt_i}", tag="ps")
                    for ct in range(4):
                        nc.tensor.matmul(ps[:], w16["q"][:, ct, ts(dt_i, 128)],
                                         x16[ic][:, ct, :],
                                         start=(ct == 0), stop=(ct == 3))
                    nc.vector.tensor_scalar_add(qt[dt_i][:, ts(ic, 512)], ps[:],
                                                bq_sb[:, dt_i:dt_i + 1])

            # ---- scores + column softmax, PV interleaved -----------------
            ep = {}

            def do_S(u):
                c0 = u // 2
                stats = persist.tile([128, 4], F32, name=f"stats{u}", tag=f"stats{u}")
                for c in range(c0, NCH):
                    ps = psum.tile([128, 512], F32, name=f"ps_s{u}{c}", tag="ps")
                    for dt_i in range(4):
                        nc.tensor.matmul(ps[:], kt[dt_i][:, ts(u, 128)],
                                         qt[dt_i][:, ts(c, 512)],
                                         start=(dt_i == 0), stop=(dt_i == 3))
                    e = ering.tile([128, 512], F16, name=f"e_{u}_{c}", tag="e")
                    ep[u, c] = e
                    if c == c0:
                        # additive causal mask (0 valid / -1e9 masked) on the
                        # PSUM scores; exp underflows to 0 on masked entries
                        m = masks["E" if u % 2 == 0 else "O"]
                        nc.vector.tensor_add(ps[:], ps[:], m[:])
                    nc.scalar.activation(e[:], ps[:], AF.Exp, bias=0.0, scale=RS,
                                         accum_out=stats[:, c:c + 1])
                zs = persist.tile([128, 1], F32, name=f"zs{u}", tag=f"zs{u}")
                nc.vector.reduce_sum(zs[:], stats[:, c0:NCH],
                                     axis=mybir.AxisListType.X)
                zi = persist.tile([128, 1], F32, name=f"zi{u}", tag=f"zi{u}")
                nc.vector.reciprocal(zi[:], zs[:])
                nc.vector.tensor_scalar_mul(vw[u][:], vw[u][:], zi[:])

            def do_PV(c):
                n_u = min(2 * c + 2, NU)
                for vb in range(4):
                    ps = psum.tile([128, 512], F32, name=f"ps_pv{c}{vb}", tag="ps")
                    for u in range(n_u):
                        nc.tensor.matmul(ps[:], vw[u][:, ts(vb, 128)], ep[u, c][:],
                                         start=(u == 0), stop=(u == n_u - 1))
                    o = outp.tile([128, 512], F32, name=f"o_{c}_{vb}", tag="o")
                    nc.vector.tensor_copy(o[:], ps[:])
                    nc.sync.dma_start(at_d[ts(vb, 128), ts(c, 512)], o[:])

            do_S(0)
            do_S(1)
            do_S(2)
            do_PV(0)
            do_S(3)
            do_S(4)
            do_PV(1)
            do_S(5)
            do_S(6)
            do_PV(2)
            do_S(7)
            do_PV(3)

    nc.compile()
    return nc


def _get_module():
    if "nc" not in _CACHE:
        _CACHE["nc"] = _build_module()
    return _CACHE["nc"]


def _host_masks(par):
    # additive masks: 0.0 where valid (i >= j), -1e9 where masked
    p = np.arange(128)[:, None]
    f = np.arange(512)[None, :]
    mE = np.where(f >= p + 128 * par, 0.0, -1e9).astype(np.float32)
    mO = np.where(f >= p + 256 + 128 * par, 0.0, -1e9).astype(np.float32)
    return mE, mO


def kernel(x, Wq, bq, Wk, bk, Wv, bv):
    from concourse.bass_utils import run_bass_kernel_spmd

    x = np.ascontiguousarray(np.asarray(x, dtype=np.float32))
    Wq16 = np.asarray(Wq, dtype=np.float16)
    Wk16 = np.asarray(Wk, dtype=np.float16)
    Wv16 = np.asarray(Wv, dtype=np.float16)
    bq = np.ascontiguousarray(np.asarray(bq, dtype=np.float32))
    bk = np.ascontiguousarray(np.asarray(bk, dtype=np.float32))
    bv = np.ascontiguousarray(np.asarray(bv, dtype=np.float32))
    x16 = x.astype(np.float16)

    nc = _get_module()

    in_maps = []
    for b in range(B):
        for par in (0, 1):
            cols = np.concatenate(
                [np.arange(128 * (2 * u + par), 128 * (2 * u + par) + 128)
                 for u in range(NU)])
            mE, mO = _host_masks(par)
            in_maps.append({
                "x16": x16[b],
                "xkv16": np.ascontiguousarray(x16[b][:, cols]),
                "wq16": Wq16, "wk16": Wk16, "wv16": Wv16,
                "bq": bq, "bk": bk, "bv": bv,
                "maskE": mE, "maskO": mO,
            })

    res = run_bass_kernel_spmd(nc, in_maps, core_ids=list(range(8)))
    _CACHE["last_results"] = res

    act = np.empty((B, VAL, S), dtype=np.float32)
    for b in range(B):
        act[b] = res.results[2 * b]["at"] + res.results[2 * b + 1]["at"]
    return np.concatenate([x, act], axis=1)


# revision 8
# speedup vs baseline: 1.4820x; 1.0804x over previous
"""Trainium2 Bass kernel for nn_AttentionBlock (B=4, C=512, S=2048, K=V=512).

Reference computation (per batch b):
  xb = x[b]                       # [C, S] channel-first
  q = xb.T @ Wq + bq              # [S, K]
  k = xb.T @ Wk + bk
  v = xb.T @ Wv + bv
  s = q @ k.T / sqrt(K)           # [Sq, Sk], causal mask j<=i
  p = softmax(s, axis=QUERY)      # normalize over i for each column j
  act = (p @ v).T                 # [V, S]
  out = concat([xb, act], axis=0) # [C+V, S]

Sharding: 8 cores = 4 batches x 2 "parity" shards. Core (b, par) owns the
interleaved key-tiles t = 2u+par (u=0..7, 128 keys each). Column-softmax
(over queries i) is fully local to a key j, so each core computes complete
softmax columns for its keys and a partial activation
  AT_par[v, q] = sum_{its j} P[j, q] * V[j, v]
The two partials per batch are summed on the host during unshard.

All on-device tensors are kept feature-major so the whole pipeline needs
zero transposes:
  QT[d, i] = Wq.T @ xb        (lhsT=Wq[c,d], rhs=xb[c,i])
  KT[d, j] = Wk.T @ xkv       (xkv = host-gathered key columns of xb)
  V [j, v] = xkv.T @ Wv       (lhsT=xkv[c,j], rhs=Wv[c,v])
  ST[j, i] = KT.T @ QT        -> softmax along the FREE axis (i) per row j
  AT[v, q] = W.T @ E          (W = V scaled by 1/Z per row j, E = exp scores)

The causal structure is identical for both parities (same chunk counts per
u), so one static program serves all 8 cores; the parity difference lives
entirely in the data (xkv gather + the two additive diagonal mask tiles).

Matmul operands are fp16 (full PE rate + fast weight load); inputs are cast
to fp16 on the host, accumulation stays fp32 in PSUM, softmax statistics in
fp32. Measured end-to-end absmax error ~2e-3 (same order as an fp32r build).
"""

import math

import numpy as np

B, C, S = 4, 512, 2048
KEY = 512
VAL = 512
NU = 8          # key-tiles (128 wide) per core
NCH = 4         # 512-wide i/q chunks
RS = 1.0 / math.sqrt(KEY)

_CACHE = {}


def _build_module():
    import concourse.bass as bass
    import concourse.tile as tile
    from concourse import bacc, mybir

    F32 = mybir.dt.float32
    F16 = mybir.dt.float16
    AF = mybir.ActivationFunctionType
    ts = bass.ts

    nc = bacc.Bacc("TRN2", target_bir_lowering=False, debug=False,
                   enable_asserts=False, num_devices=8)

    x_d = nc.dram_tensor("x16", [C, S], F16, kind="ExternalInput").ap()
    xkv_d = nc.dram_tensor("xkv16", [C, NU * 128], F16, kind="ExternalInput").ap()
    wq_d = nc.dram_tensor("wq16", [C, KEY], F16, kind="ExternalInput").ap()
    wk_d = nc.dram_tensor("wk16", [C, KEY], F16, kind="ExternalInput").ap()
    wv_d = nc.dram_tensor("wv16", [C, VAL], F16, kind="ExternalInput").ap()
    bq_d = nc.dram_tensor("bq", [KEY], F32, kind="ExternalInput").ap()
    bk_d = nc.dram_tensor("bk", [KEY], F32, kind="ExternalInput").ap()
    bv_d = nc.dram_tensor("bv", [VAL], F32, kind="ExternalInput").ap()
    me_d = nc.dram_tensor("maskE", [128, 512], F32, kind="ExternalInput").ap()
    mo_d = nc.dram_tensor("maskO", [128, 512], F32, kind="ExternalInput").ap()
    at_d = nc.dram_tensor("at", [VAL, S], F32, kind="ExternalOutput").ap()

    # dram views with the 512-row axis split into 4 partition tiles
    x_v = x_d.rearrange("(ct p) s -> p ct s", p=128)
    xkv_v = xkv_d.rearrange("(ct p) s -> p ct s", p=128)
    wq_v = wq_d.rearrange("(ct p) d -> p ct d", p=128)
    wk_v = wk_d.rearrange("(ct p) d -> p ct d", p=128)
    wv_v = wv_d.rearrange("(ct p) d -> p ct d", p=128)
    bq_v = bq_d.rearrange("(dt p) -> p dt", p=128)
    bk_v = bk_d.rearrange("(dt p) -> p dt", p=128)

    with tile.TileContext(nc) as tc:
        with tc.tile_pool(name="persist", bufs=1) as persist, \
             tc.tile_pool(name="ering", bufs=20) as ering, \
             tc.tile_pool(name="outp", bufs=4) as outp, \
             tc.tile_pool(name="psum", bufs=6, space="PSUM") as psum:

            # ---- inputs: weights/xkv on scalar queue, x on sync queue ----
            w16 = {}
            for nm, v in (("k", wk_v), ("v", wv_v), ("q", wq_v)):
                t = persist.tile([128, 4, 512], F16, name=f"w{nm}", tag=f"w{nm}")
                nc.scalar.dma_start(t[:], v)
                w16[nm] = t
            xkv16 = persist.tile([128, 4, 1024], F16, name="xkv16s", tag="xkv")
            for jc in range(2):
                nc.sync.dma_start(xkv16[:, :, ts(jc, 512)], xkv_v[:, :, ts(jc, 512)])
            x16 = []
            for ic in range(NCH):
                t = persist.tile([128, 4, 512], F16, name=f"x16_{ic}", tag=f"x{ic}")
                eng = nc.sync if ic < 2 else nc.scalar
                eng.dma_start(t[:], x_v[:, :, ts(ic, 512)])
                x16.append(t)

            masks = {}
            for nm, d in (("E", me_d), ("O", mo_d)):
                m = persist.tile([128, 512], F32, name=f"mask{nm}", tag=f"mask{nm}")
                nc.scalar.dma_start(m[:], d)
                masks[nm] = m
            bq_sb = persist.tile([128, 4], F32, name="bq_sb", tag="bq_sb")
            nc.scalar.dma_start(bq_sb[:], bq_v)
            bk_sb = persist.tile([128, 4], F32, name="bk_sb", tag="bk_sb")
            nc.scalar.dma_start(bk_sb[:], bk_v)
            bvb = persist.tile([128, 512], F32, name="bvb", tag="bvb")
            bv_bcast = bass.AP(tensor=bv_d.tensor, offset=bv_d.offset,
                               ap=[[0, 128]] + list(bv_d.ap))
            nc.gpsimd.dma_start(bvb[:], bv_bcast)

            # ---- projections: KT[d, j], V[j, v] (+bv), QT[d, i] ----------
            # order interleaved so the second xkv half's DMA is covered by
            # compute on the first half
            kt = [persist.tile([128, 1024], F16, name=f"kt{d}", tag=f"kt{d}")
                  for d in range(4)]
            vw = [persist.tile([128, 512], F16, name=f"vw{u}", tag=f"vw{u}")
                  for u in range(NU)]
            qt = [persist.tile([128, 2048], F16, name=f"qt{d}", tag=f"qt{d}")
                  for d in range(4)]

            def do_KT(jc):
                for dt_i in range(4):
                    ps = psum.tile([128, 512], F32, name=f"ps_kt{jc}{dt_i}", tag="ps")
                    for ct in range(4):
                        nc.tensor.matmul(ps[:], w16["k"][:, ct, ts(dt_i, 128)],
                                         xkv16[:, ct, ts(jc, 512)],
                                         start=(ct == 0), stop=(ct == 3))
                    nc.vector.tensor_scalar_add(kt[dt_i][:, ts(jc, 512)], ps[:],
                                                bk_sb[:, dt_i:dt_i + 1])

            def do_V(u):
                ps = psum.tile([128, 512], F32, name=f"ps_v{u}", tag="ps")
                for ct in range(4):
                    nc.tensor.matmul(ps[:], xkv16[:, ct, ts(u, 128)],
                                     w16["v"][:, ct, :],
                                     start=(ct == 0), stop=(ct == 3))
                nc.vector.tensor_add(vw[u][:], ps[:], bvb[:])

            def do_QT(ic):
                for dt_i in range(4):
                    ps = psum.tile([128, 512], F32, name=f"ps_qt{ic}{dt_i}", tag="ps")
                    for ct in range(4):
                        nc.tensor.matmul(ps[:], w16["q"][:, ct, ts(dt_i, 128)],
                                         x16[ic][:, ct, :],
                                         start=(ct == 0), stop=(ct == 3))
                    nc.scalar.activation(qt[dt_i][:, ts(ic, 512)], ps[:],
                                         AF.Identity, bias=bq_sb[:, dt_i:dt_i + 1],
                                         scale=1.0)

            do_KT(0)
            for u in range(4):
                do_V(u)
            do_KT(1)
            for u in range(4, 8):
                do_V(u)
            for ic in range(NCH):
                do_QT(ic)

            # ---- scores + column softmax, PV interleaved -----------------
            ep = {}

            def do_S(u):
                c0 = u // 2
                stats = persist.tile([128, 4], F32, name=f"stats{u}", tag=f"stats{u}")
                for c in range(c0, NCH):
                    ps = psum.tile([128, 512], F32, name=f"ps_s{u}{c}", tag="ps")
                    for dt_i in range(4):
                        nc.tensor.matmul(ps[:], kt[dt_i][:, ts(u, 128)],
                                         qt[dt_i][:, ts(c, 512)],
                                         start=(dt_i == 0), stop=(dt_i == 3))
                    e = ering.tile([128, 512], F16, name=f"e_{u}_{c}", tag="e")
                    ep[u, c] = e
                    if c == c0:
                        # additive causal mask (0 valid / -1e9 masked) on the
                        # PSUM scores; exp underflows to 0 on masked entries
                        m = masks["E" if u % 2 == 0 else "O"]
                        nc.vector.tensor_add(ps[:], ps[:], m[:])
                    nc.scalar.activation(e[:], ps[:], AF.Exp, bias=0.0, scale=RS,
                                         accum_out=stats[:, c:c + 1])
                zs = persist.tile([128, 1], F32, name=f"zs{u}", tag=f"zs{u}")
                nc.vector.reduce_sum(zs[:], stats[:, c0:NCH],
                                     axis=mybir.AxisListType.X)
                zi = persist.tile([128, 1], F32, name=f"zi{u}", tag=f"zi{u}")
                nc.vector.reciprocal(zi[:], zs[:])
                nc.vector.tensor_scalar_mul(vw[u][:], vw[u][:], zi[:])

            def do_PV(c):
                n_u = min(2 * c + 2, NU)
                for vb in range(4):
                    ps = psum.tile([128, 512], F32, name=f"ps_pv{c}{vb}", tag="ps")
                    for u in range(n_u):
                        nc.tensor.matmul(ps[:], vw[u][:, ts(vb, 128)], ep[u, c][:],
                                         start=(u == 0), stop=(u == n_u - 1))
                    o = outp.tile([128, 512], F32, name=f"o_{c}_{vb}", tag="o")
                    nc.vector.tensor_copy(o[:], ps[:])
                    nc.sync.dma_start(at_d[ts(vb, 128), ts(c, 512)], o[:])

            do_S(0)
            do_S(1)
            do_S(2)
            do_PV(0)
            do_S(3)
            do_S(4)
            do_PV(1)
            do_S(5)
            do_S(6)
            do_PV(2)
            do_S(7)
            do_PV(3)

    nc.compile()
    return nc


def _get_module():
    if "nc" not in _CACHE:
        _CACHE["nc"] = _build_module()
    return _CACHE["nc"]


def _host_masks(par):
    # additive masks: 0.0 where valid (i >= j), -1e9 where masked
    p = np.arange(128)[:, None]
    f = np.arange(512)[None, :]
    mE = np.where(f >= p + 128 * par, 0.0, -1e9).astype(np.float32)
    mO = np.where(f >= p + 256 + 128 * par, 0.0, -1e9).astype(np.float32)
    return mE, mO


def kernel(x, Wq, bq, Wk, bk, Wv, bv):
    from concourse.bass_utils import run_bass_kernel_spmd

    x = np.ascontiguousarray(np.asarray(x, dtype=np.float32))
    Wq16 = np.asarray(Wq, dtype=np.float16)
    Wk16 = np.asarray(Wk, dtype=np.float16)
    Wv16 = np.asarray(Wv, dtype=np.float16)
    bq = np.ascontiguousarray(np.asarray(bq, dtype=np.float32))
    bk = np.ascontiguousarray(np.asarray(bk, dtype=np.float32))
    bv = np.ascontiguousarray(np.asarray(bv, dtype=np.float32))
    x16 = x.astype(np.float16)

    nc = _get_module()

    in_maps = []
    for b in range(B):
        for par in (0, 1):
            cols = np.concatenate(
                [np.arange(128 * (2 * u + par), 128 * (2 * u + par) + 128)
                 for u in range(NU)])
            mE, mO = _host_masks(par)
            in_maps.append({
                "x16": x16[b],
                "xkv16": np.ascontiguousarray(x16[b][:, cols]),
                "wq16": Wq16, "wk16": Wk16, "wv16": Wv16,
                "bq": bq, "bk": bk, "bv": bv,
                "maskE": mE, "maskO": mO,
            })

    res = run_bass_kernel_spmd(nc, in_maps, core_ids=list(range(8)))
    _CACHE["last_results"] = res

    act = np.empty((B, VAL, S), dtype=np.float32)
    for b in range(B):
        act[b] = res.results[2 * b]["at"] + res.results[2 * b + 1]["at"]
    return np.concatenate([x, act], axis=1)
